# revision 37
# baseline (speedup 1.0000x reference)
"""Conv3d(16->64, k=3, VALID) + sigmoid(tanh(conv*scale)*bias), B=8 sharded
over 8 NeuronCores (one batch element per core).

v2 scheme (per core), 4 matmul passes per output tile:
  - x is pre-cast to bf16 on host, padded with garbage planes at the end.
  - tileA [128 = (kd,kw)-combos x ci, C-plane window]: 8 of the 9 (kd,kw)
    shifts baked into partitions (all but (2,2)); kh handled as free-dim
    offsets. 3 passes of K=128 cover 24 of 27 kernel taps.
  - tile48 [48 = (kh,ci), (C+2)-plane window]: kh baked into partitions;
    one K=48 pass with free offset (kd=2, kw=2) covers the last 3 taps.
  - 4 matmuls accumulate into a [64, 2048] PSUM half-plane (scale folded
    into the weights on host).
  - Epilogue on ACT: tanh then sigmoid(bias * t) with per-partition bias
    as the activation scale operand; store valid [64, h, 62] rows to HBM.
"""

import sys

sys.path.insert(0, "/opt/trn_rl_repo")

import numpy as np
import ml_dtypes

import concourse.bass as bass
import concourse.mybir as mybir
from concourse import tile
from concourse.bass_utils import run_bass_kernel_spmd

# ---- problem constants (hardcoded per spec) ----
B = 8
CIN = 16
COUT = 64
K = 3
S = 64  # input spatial
SO = S - K + 1  # 62 output spatial
PLANE = S * S  # 4096
HALF = PLANE // 2  # 2048
NCORES = 8

CHUNK = 4  # output planes per window load

PAD_PLANES = 7
XPLANES = S + PAD_PLANES
XSTRIDE = XPLANES * PLANE  # per-channel row stride in xp

WINA = CHUNK * PLANE + 160  # tileA window elements per partition
WIN48 = CHUNK * PLANE + 64  # tile48 window elements per partition


def split_multiwaits(nc):
    """walrus in this toolchain rejects instructions carrying more than one
    sync-wait. Rewrite every multi-wait instruction into (n-1) single-wait
    nops on the same engine queue followed by the instruction with the last
    wait — identical semantics since each engine queue executes serially."""
    for func in nc.m.functions:
        for block in func.blocks:
            insts = block.instructions
            if not any(
                i.sync_info is not None and len(i.sync_info.on_wait or ()) > 1
                for i in insts
            ):
                continue
            newlist = []
            for inst in insts:
                si = inst.sync_info
                if si is not None and si.on_wait and len(si.on_wait) > 1:
                    waits = list(si.on_wait)
                    for w in waits[:-1]:
                        nop = mybir.InstNoOp(
                            name=nc.get_next_instruction_name(),
                            sync_info=mybir.SyncInfo(on_wait=[w], on_update=[]),
                            bass_nofuse=True,
                            engine=inst.engine,
                        )
                        newlist.append(nop)
                    si.on_wait = waits[-1:]
                newlist.append(inst)
            insts[:] = newlist


class PatchedTileContext(tile.TileContext):
    def __exit__(self, exc_type, exc_value, traceback):
        ret = super().__exit__(exc_type, exc_value, traceback)
        if exc_type is None:
            split_multiwaits(self.nc)
        return ret


def build_nc(nplanes=SO, repeat=1, ablate=()):
    CHUNK_STARTS = list(range(0, nplanes, CHUNK))
    nc = bass.Bass(trn_type="TRN2")
    # host-replicated x: row (kd*48 + kw*16 + ci) = x[ci] shifted kd*PLANE+kw
    xr = nc.dram_tensor(
        "xr", [128, XPLANES * PLANE], mybir.dt.bfloat16, kind="ExternalInput"
    )
    # host-replicated x: row (kh*16 + ci) = x[ci] shifted kh*S
    xr48 = nc.dram_tensor(
        "xr48", [K * CIN, XPLANES * PLANE], mybir.dt.bfloat16, kind="ExternalInput"
    )
    wla = nc.dram_tensor("wla", [128, K * COUT], mybir.dt.bfloat16, kind="ExternalInput")
    wl48 = nc.dram_tensor("wl48", [K * CIN, COUT], mybir.dt.bfloat16, kind="ExternalInput")
    bv = nc.dram_tensor("bv", [2 * COUT, 1], mybir.dt.float32, kind="ExternalInput")
    out = nc.dram_tensor(
        "out", [COUT, SO * SO * SO], mybir.dt.float32, kind="ExternalOutput"
    )

    fp32 = mybir.dt.float32
    bf16 = mybir.dt.bfloat16
    AF = mybir.ActivationFunctionType

    with PatchedTileContext(nc) as tc:
        with (
            tc.tile_pool(name="const", bufs=1) as cpool,
            tc.tile_pool(name="xwina", bufs=2) as xapool,
            tc.tile_pool(name="xwin48", bufs=2) as x48pool,
            tc.tile_pool(name="eptmp", bufs=2) as epool,
            tc.tile_pool(name="outp", bufs=2) as opool,
            tc.tile_pool(name="psum", bufs=2, space="PSUM") as pspool,
        ):
            wla_sb = cpool.tile([128, K * COUT], bf16)
            wl48_sb = cpool.tile([K * CIN, COUT], bf16)
            bv_sb = cpool.tile([2 * COUT, 1], fp32)
            nc.sync.dma_start(wla_sb[:], wla[:])
            nc.sync.dma_start(wl48_sb[:], wl48[:])
            nc.sync.dma_start(bv_sb[:], bv[:])

            def body():
                for d0 in CHUNK_STARTS:
                    run_chunk(d0)

            def run_chunk(d0):
                ndp = min(CHUNK, SO - d0)  # output planes this chunk
                # single full-width DMA per window: 128 partitions = all ports
                xa = xapool.tile([128, WINA], bf16, tag="xa")
                nc.sync.dma_start(
                    xa[:], xr[:, d0 * PLANE : d0 * PLANE + WINA]
                )
                # tile48 window starts at d0+2 (only the kd=2,kw=2 tap reads it);
                # issued on the Pool/SWDGE queue to keep SP free
                x48 = x48pool.tile([K * CIN, WIN48], bf16, tag="x48")
                nc.gpsimd.dma_start(
                    x48[:],
                    xr48[:, (d0 + 2) * PLANE : (d0 + 2) * PLANE + WIN48],
                )

                for dl in range(ndp):
                    d = d0 + dl
                    # plane halves land on PSUM partition halves: p = 64*half+co
                    # pass-outer order: 8 consecutive matmuls share one lhsT
                    ps = pspool.tile([2 * COUT, HALF], fp32, tag="ps")
                    if "mm" in ablate:
                        nc.tensor.matmul(
                            ps[0:COUT, 0:512],
                            wla_sb[:, 0:COUT],
                            xa[:, 0:512],
                            start=True,
                            stop=True,
                        )
                    else:
                        # half innermost: consecutive matmuls alternate PE
                        # column groups (PSUM partitions 0:64 / 64:128) so the
                        # 32x32 sub-arrays can overlap execution
                        for kh in range(K):
                            for b in range(4):
                                for half in range(2):
                                    po = half * COUT
                                    col = half * HALF + b * 512
                                    oa = dl * PLANE + kh * S + col
                                    nc.tensor.matmul(
                                        ps[po : po + COUT, b * 512 : (b + 1) * 512],
                                        wla_sb[:, kh * COUT : (kh + 1) * COUT],
                                        xa[:, oa : oa + 512],
                                        start=(kh == 0),
                                        stop=False,
                                    )
                        for b in range(4):
                            for half in range(2):
                                po = half * COUT
                                col = half * HALF + b * 512
                                o48 = dl * PLANE + 2 + col
                                nc.tensor.matmul(
                                    ps[po : po + COUT, b * 512 : (b + 1) * 512],
                                    wl48_sb[:],
                                    x48[:, o48 : o48 + 512],
                                    start=False,
                                    stop=True,
                                )
                    # 128-partition epilogue: both halves in one ACT pass
                    t_sb = epool.tile([2 * COUT, HALF], fp32, tag="t")
                    o_sb = opool.tile([2 * COUT, HALF], fp32, tag="o")
                    if "act" not in ablate:
                        nc.scalar.activation(t_sb[:], ps[:], AF.Tanh)
                        nc.scalar.activation(
                            o_sb[:], t_sb[:], AF.Sigmoid, scale=bv_sb[:]
                        )
                    else:
                        nc.scalar.activation(o_sb[:, 0:512], ps[:, 0:512], AF.Tanh)
                    # two stores per plane, split across the SP and ACT HWDGE queues
                    src0 = o_sb[0:COUT, :].rearrange("p (h w) -> p h w", h=32)[
                        :, :, :SO
                    ]
                    nc.sync.dma_start(
                        out[:, d * SO * SO : d * SO * SO + 32 * SO], src0
                    )
                    src1 = o_sb[COUT : 2 * COUT, :].rearrange("p (h w) -> p h w", h=32)[
                        :, : SO - 32, :SO
                    ]
                    nc.scalar.dma_start(
                        out[:, d * SO * SO + 32 * SO : (d + 1) * SO * SO], src1
                    )

            for _ in range(repeat):
                body()
    return nc


_NC_CACHE = None
LAST_RESULT = None


def kernel(x, weight, scale, bias):
    global _NC_CACHE, LAST_RESULT
    x = np.asarray(x, dtype=np.float32)
    weight = np.asarray(weight, dtype=np.float32)
    scale = np.asarray(scale, dtype=np.float32)
    bias = np.asarray(bias, dtype=np.float32)

    # fold scale into weights
    w_eff = weight * scale.reshape(COUT, 1, 1, 1, 1)  # [co, ci, kd, kh, kw]

    # wla: [128 = (kd,kw,ci) packed, kh*64 + co]
    wla = np.zeros((128, K * COUT), dtype=np.float32)
    wt = w_eff.transpose(2, 4, 1, 3, 0)  # [kd, kw, ci, kh, co]
    wla[0:96] = wt[0:2].reshape(96, K * COUT)
    wla[96:128] = wt[2, 0:2].reshape(32, K * COUT)
    wla = wla.astype(ml_dtypes.bfloat16)

    # wl48: [48 = (kh,ci), co] for tap (kd=2, kw=2)
    wl48 = np.ascontiguousarray(
        w_eff[:, :, 2, :, 2].transpose(2, 1, 0).reshape(K * CIN, COUT)
    ).astype(ml_dtypes.bfloat16)

    bv = np.ascontiguousarray(
        np.concatenate([bias.reshape(COUT, 1)] * 2, axis=0)
    )

    # host-side cast to bf16 + shift-replication into the matmul layouts
    xf = x.reshape(B, CIN, S * PLANE).astype(ml_dtypes.bfloat16)
    NTOT = S * PLANE
    xr = np.zeros((B, 128, XPLANES * PLANE), dtype=ml_dtypes.bfloat16)
    for kd in range(K):
        for kw in range(K):
            if kd == 2 and kw == 2:
                continue
            p = kd * 48 + kw * 16 if kd < 2 else 96 + kw * 16
            sh = kd * PLANE + kw
            xr[:, p : p + CIN, : NTOT - sh] = xf[:, :, sh:]
    xr48 = np.zeros((B, K * CIN, XPLANES * PLANE), dtype=ml_dtypes.bfloat16)
    for kh in range(K):
        sh = kh * S
        xr48[:, kh * CIN : (kh + 1) * CIN, : NTOT - sh] = xf[:, :, sh:]

    if _NC_CACHE is None:
        _NC_CACHE = build_nc()
    nc = _NC_CACHE

    in_maps = [
        {"xr": xr[c], "xr48": xr48[c], "wla": wla, "wl48": wl48, "bv": bv}
        for c in range(NCORES)
    ]
    res = run_bass_kernel_spmd(nc, in_maps, list(range(NCORES)))
    LAST_RESULT = res

    out = np.empty((B, COUT, SO, SO, SO), dtype=np.float32)
    for c in range(NCORES):
        out[c] = res.results[c]["out"].reshape(COUT, SO, SO, SO)
    return out


# revision 40
# speedup vs baseline: 3.6557x; 3.6557x over previous
"""Conv3d(16->64, k=3, VALID) + sigmoid(tanh(conv*scale)*bias), B=8 sharded
over 8 NeuronCores (one batch element per core).

v2 scheme (per core), 4 matmul passes per output tile:
  - x is pre-cast to bf16 on host, padded with garbage planes at the end.
  - tileA [128 = (kd,kw)-combos x ci, C-plane window]: 8 of the 9 (kd,kw)
    shifts baked into partitions (all but (2,2)); kh handled as free-dim
    offsets. 3 passes of K=128 cover 24 of 27 kernel taps.
  - tile48 [48 = (kh,ci), (C+2)-plane window]: kh baked into partitions;
    one K=48 pass with free offset (kd=2, kw=2) covers the last 3 taps.
  - 4 matmuls accumulate into a [64, 2048] PSUM half-plane (scale folded
    into the weights on host).
  - Epilogue on ACT: tanh then sigmoid(bias * t) with per-partition bias
    as the activation scale operand; store valid [64, h, 62] rows to HBM.
"""

import sys

sys.path.insert(0, "/opt/trn_rl_repo")

import numpy as np
import ml_dtypes

import concourse.bass as bass
import concourse.mybir as mybir
from concourse import tile
from concourse.bass_utils import run_bass_kernel_spmd

# ---- problem constants (hardcoded per spec) ----
B = 8
CIN = 16
COUT = 64
K = 3
S = 64  # input spatial
SO = S - K + 1  # 62 output spatial
PLANE = S * S  # 4096
HALF = PLANE // 2  # 2048
NCORES = 8

CHUNK = 4  # output planes per window load

PAD_PLANES = 7
XPLANES = S + PAD_PLANES
XSTRIDE = XPLANES * PLANE  # per-channel row stride in xp

WINA = CHUNK * PLANE + 160  # tileA window elements per partition
WIN48 = CHUNK * PLANE + 64  # tile48 window elements per partition


def split_multiwaits(nc):
    """walrus in this toolchain rejects instructions carrying more than one
    sync-wait. Rewrite every multi-wait instruction into (n-1) single-wait
    nops on the same engine queue followed by the instruction with the last
    wait — identical semantics since each engine queue executes serially."""
    for func in nc.m.functions:
        for block in func.blocks:
            insts = block.instructions
            if not any(
                i.sync_info is not None and len(i.sync_info.on_wait or ()) > 1
                for i in insts
            ):
                continue
            newlist = []
            for inst in insts:
                si = inst.sync_info
                if si is not None and si.on_wait and len(si.on_wait) > 1:
                    waits = list(si.on_wait)
                    for w in waits[:-1]:
                        nop = mybir.InstNoOp(
                            name=nc.get_next_instruction_name(),
                            sync_info=mybir.SyncInfo(on_wait=[w], on_update=[]),
                            bass_nofuse=True,
                            engine=inst.engine,
                        )
                        newlist.append(nop)
                    si.on_wait = waits[-1:]
                newlist.append(inst)
            insts[:] = newlist


class PatchedTileContext(tile.TileContext):
    def __exit__(self, exc_type, exc_value, traceback):
        ret = super().__exit__(exc_type, exc_value, traceback)
        if exc_type is None:
            split_multiwaits(self.nc)
        return ret


def build_nc(nplanes=SO, repeat=1, ablate=()):
    CHUNK_STARTS = list(range(0, nplanes, CHUNK))
    nc = bass.Bass(trn_type="TRN2")
    # host-replicated x: row (kd*48 + kw*16 + ci) = x[ci] shifted kd*PLANE+kw
    xr = nc.dram_tensor(
        "xr", [128, XPLANES * PLANE], mybir.dt.bfloat16, kind="ExternalInput"
    )
    # host-replicated x: row (kh*16 + ci) = x[ci] shifted kh*S
    xr48 = nc.dram_tensor(
        "xr48", [K * CIN, XPLANES * PLANE], mybir.dt.bfloat16, kind="ExternalInput"
    )
    wla = nc.dram_tensor("wla", [128, K * COUT], mybir.dt.bfloat16, kind="ExternalInput")
    wl48 = nc.dram_tensor("wl48", [K * CIN, COUT], mybir.dt.bfloat16, kind="ExternalInput")
    bv = nc.dram_tensor("bv", [2 * COUT, 1], mybir.dt.float32, kind="ExternalInput")
    # full 64x64 (h,w) planes incl. garbage columns — contiguous stores;
    # the valid [.., :62, :62] crop happens on the host
    out = nc.dram_tensor(
        "out", [COUT, SO * PLANE], mybir.dt.float32, kind="ExternalOutput"
    )

    fp32 = mybir.dt.float32
    bf16 = mybir.dt.bfloat16
    AF = mybir.ActivationFunctionType

    with PatchedTileContext(nc) as tc:
        with (
            tc.tile_pool(name="const", bufs=1) as cpool,
            tc.tile_pool(name="xwina", bufs=2) as xapool,
            tc.tile_pool(name="xwin48", bufs=2) as x48pool,
            tc.tile_pool(name="eptmp", bufs=2) as epool,
            tc.tile_pool(name="outp", bufs=2) as opool,
            tc.tile_pool(name="psum", bufs=2, space="PSUM") as pspool,
        ):
            wla_sb = cpool.tile([128, K * COUT], bf16)
            wl48_sb = cpool.tile([K * CIN, COUT], bf16)
            bv_sb = cpool.tile([2 * COUT, 1], fp32)
            nc.sync.dma_start(wla_sb[:], wla[:])
            nc.sync.dma_start(wl48_sb[:], wl48[:])
            nc.sync.dma_start(bv_sb[:], bv[:])

            def body():
                for d0 in CHUNK_STARTS:
                    run_chunk(d0)

            def run_chunk(d0):
                ndp = min(CHUNK, SO - d0)  # output planes this chunk
                # single full-width DMA per window: 128 partitions = all ports
                xa = xapool.tile([128, WINA], bf16, tag="xa")
                nc.sync.dma_start(
                    xa[:], xr[:, d0 * PLANE : d0 * PLANE + WINA]
                )
                # tile48 window starts at d0+2 (only the kd=2,kw=2 tap reads it);
                # issued on the Pool/SWDGE queue to keep SP free
                x48 = x48pool.tile([K * CIN, WIN48], bf16, tag="x48")
                nc.gpsimd.dma_start(
                    x48[:],
                    xr48[:, (d0 + 2) * PLANE : (d0 + 2) * PLANE + WIN48],
                )

                for dl in range(ndp):
                    d = d0 + dl
                    # plane halves land on PSUM partition halves: p = 64*half+co
                    # pass-outer order: 8 consecutive matmuls share one lhsT
                    ps = pspool.tile([2 * COUT, HALF], fp32, tag="ps")
                    if "mm" in ablate:
                        nc.tensor.matmul(
                            ps[0:COUT, 0:512],
                            wla_sb[:, 0:COUT],
                            xa[:, 0:512],
                            start=True,
                            stop=True,
                        )
                    else:
                        # half innermost: consecutive matmuls alternate PE
                        # column groups (PSUM partitions 0:64 / 64:128) so the
                        # 32x32 sub-arrays can overlap execution
                        for kh in range(K):
                            for b in range(4):
                                for half in range(2):
                                    po = half * COUT
                                    col = half * HALF + b * 512
                                    oa = dl * PLANE + kh * S + col
                                    nc.tensor.matmul(
                                        ps[po : po + COUT, b * 512 : (b + 1) * 512],
                                        wla_sb[:, kh * COUT : (kh + 1) * COUT],
                                        xa[:, oa : oa + 512],
                                        start=(kh == 0),
                                        stop=False,
                                    )
                        for b in range(4):
                            for half in range(2):
                                po = half * COUT
                                col = half * HALF + b * 512
                                o48 = dl * PLANE + 2 + col
                                nc.tensor.matmul(
                                    ps[po : po + COUT, b * 512 : (b + 1) * 512],
                                    wl48_sb[:],
                                    x48[:, o48 : o48 + 512],
                                    start=False,
                                    stop=True,
                                )
                    # 128-partition epilogue: both halves in one ACT pass
                    t_sb = epool.tile([2 * COUT, HALF], fp32, tag="t")
                    o_sb = opool.tile([2 * COUT, HALF], fp32, tag="o")
                    if "act" not in ablate:
                        nc.scalar.activation(t_sb[:], ps[:], AF.Tanh)
                        nc.scalar.activation(
                            o_sb[:], t_sb[:], AF.Sigmoid, scale=bv_sb[:]
                        )
                    else:
                        nc.scalar.activation(o_sb[:, 0:512], ps[:, 0:512], AF.Tanh)
                    # two fully-contiguous stores per plane, split across the
                    # SP and ACT HWDGE queues (descriptor-light)
                    nc.sync.dma_start(
                        out[:, d * PLANE : d * PLANE + HALF], o_sb[0:COUT, :]
                    )
                    nc.scalar.dma_start(
                        out[:, d * PLANE + HALF : (d + 1) * PLANE],
                        o_sb[COUT : 2 * COUT, :],
                    )

            for _ in range(repeat):
                body()
    return nc


_NC_CACHE = None
LAST_RESULT = None


def kernel(x, weight, scale, bias):
    global _NC_CACHE, LAST_RESULT
    x = np.asarray(x, dtype=np.float32)
    weight = np.asarray(weight, dtype=np.float32)
    scale = np.asarray(scale, dtype=np.float32)
    bias = np.asarray(bias, dtype=np.float32)

    # fold scale into weights
    w_eff = weight * scale.reshape(COUT, 1, 1, 1, 1)  # [co, ci, kd, kh, kw]

    # wla: [128 = (kd,kw,ci) packed, kh*64 + co]
    wla = np.zeros((128, K * COUT), dtype=np.float32)
    wt = w_eff.transpose(2, 4, 1, 3, 0)  # [kd, kw, ci, kh, co]
    wla[0:96] = wt[0:2].reshape(96, K * COUT)
    wla[96:128] = wt[2, 0:2].reshape(32, K * COUT)
    wla = wla.astype(ml_dtypes.bfloat16)

    # wl48: [48 = (kh,ci), co] for tap (kd=2, kw=2)
    wl48 = np.ascontiguousarray(
        w_eff[:, :, 2, :, 2].transpose(2, 1, 0).reshape(K * CIN, COUT)
    ).astype(ml_dtypes.bfloat16)

    bv = np.ascontiguousarray(
        np.concatenate([bias.reshape(COUT, 1)] * 2, axis=0)
    )

    # host-side cast to bf16 + shift-replication into the matmul layouts
    xf = x.reshape(B, CIN, S * PLANE).astype(ml_dtypes.bfloat16)
    NTOT = S * PLANE
    xr = np.zeros((B, 128, XPLANES * PLANE), dtype=ml_dtypes.bfloat16)
    for kd in range(K):
        for kw in range(K):
            if kd == 2 and kw == 2:
                continue
            p = kd * 48 + kw * 16 if kd < 2 else 96 + kw * 16
            sh = kd * PLANE + kw
            xr[:, p : p + CIN, : NTOT - sh] = xf[:, :, sh:]
    xr48 = np.zeros((B, K * CIN, XPLANES * PLANE), dtype=ml_dtypes.bfloat16)
    for kh in range(K):
        sh = kh * S
        xr48[:, kh * CIN : (kh + 1) * CIN, : NTOT - sh] = xf[:, :, sh:]

    if _NC_CACHE is None:
        _NC_CACHE = build_nc()
    nc = _NC_CACHE

    in_maps = [
        {"xr": xr[c], "xr48": xr48[c], "wla": wla, "wl48": wl48, "bv": bv}
        for c in range(NCORES)
    ]
    res = run_bass_kernel_spmd(nc, in_maps, list(range(NCORES)))
    LAST_RESULT = res

    out = np.empty((B, COUT, SO, SO, SO), dtype=np.float32)
    for c in range(NCORES):
        full = res.results[c]["out"].reshape(COUT, SO, S, S)
        out[c] = full[:, :, :SO, :SO]
    return out


# revision 41
# speedup vs baseline: 5.8534x; 1.6012x over previous
"""Conv3d(16->64, k=3, VALID) + sigmoid(tanh(conv*scale)*bias), B=8 sharded
over 8 NeuronCores (one batch element per core).

v2 scheme (per core), 4 matmul passes per output tile:
  - x is pre-cast to bf16 on host, padded with garbage planes at the end.
  - tileA [128 = (kd,kw)-combos x ci, C-plane window]: 8 of the 9 (kd,kw)
    shifts baked into partitions (all but (2,2)); kh handled as free-dim
    offsets. 3 passes of K=128 cover 24 of 27 kernel taps.
  - tile48 [48 = (kh,ci), (C+2)-plane window]: kh baked into partitions;
    one K=48 pass with free offset (kd=2, kw=2) covers the last 3 taps.
  - 4 matmuls accumulate into a [64, 2048] PSUM half-plane (scale folded
    into the weights on host).
  - Epilogue on ACT: tanh then sigmoid(bias * t) with per-partition bias
    as the activation scale operand; store valid [64, h, 62] rows to HBM.
"""

import sys

sys.path.insert(0, "/opt/trn_rl_repo")

import numpy as np
import ml_dtypes

import concourse.bass as bass
import concourse.mybir as mybir
from concourse import tile
from concourse.bass_utils import run_bass_kernel_spmd

# ---- problem constants (hardcoded per spec) ----
B = 8
CIN = 16
COUT = 64
K = 3
S = 64  # input spatial
SO = S - K + 1  # 62 output spatial
PLANE = S * S  # 4096
HALF = PLANE // 2  # 2048
NCORES = 8

CHUNK = 4  # output planes per window load

PAD_PLANES = 7
XPLANES = S + PAD_PLANES
XSTRIDE = XPLANES * PLANE  # per-channel row stride in xp

WINA = CHUNK * PLANE + 160  # tileA window elements per partition
WIN48 = CHUNK * PLANE + 64  # tile48 window elements per partition


def split_multiwaits(nc):
    """walrus in this toolchain rejects instructions carrying more than one
    sync-wait. Rewrite every multi-wait instruction into (n-1) single-wait
    nops on the same engine queue followed by the instruction with the last
    wait — identical semantics since each engine queue executes serially."""
    for func in nc.m.functions:
        for block in func.blocks:
            insts = block.instructions
            if not any(
                i.sync_info is not None and len(i.sync_info.on_wait or ()) > 1
                for i in insts
            ):
                continue
            newlist = []
            for inst in insts:
                si = inst.sync_info
                if si is not None and si.on_wait and len(si.on_wait) > 1:
                    waits = list(si.on_wait)
                    for w in waits[:-1]:
                        nop = mybir.InstNoOp(
                            name=nc.get_next_instruction_name(),
                            sync_info=mybir.SyncInfo(on_wait=[w], on_update=[]),
                            bass_nofuse=True,
                            engine=inst.engine,
                        )
                        newlist.append(nop)
                    si.on_wait = waits[-1:]
                newlist.append(inst)
            insts[:] = newlist


class PatchedTileContext(tile.TileContext):
    def __exit__(self, exc_type, exc_value, traceback):
        ret = super().__exit__(exc_type, exc_value, traceback)
        if exc_type is None:
            split_multiwaits(self.nc)
        return ret


def build_nc(nplanes=SO, repeat=1, ablate=()):
    CHUNK_STARTS = list(range(0, nplanes, CHUNK))
    nc = bass.Bass(trn_type="TRN2")
    # host-replicated x: row (kd*48 + kw*16 + ci) = x[ci] shifted kd*PLANE+kw
    xr = nc.dram_tensor(
        "xr", [128, XPLANES * PLANE], mybir.dt.bfloat16, kind="ExternalInput"
    )
    # host-replicated x: row (kh*16 + ci) = x[ci] shifted kh*S
    xr48 = nc.dram_tensor(
        "xr48", [K * CIN, XPLANES * PLANE], mybir.dt.bfloat16, kind="ExternalInput"
    )
    wla = nc.dram_tensor("wla", [128, K * COUT], mybir.dt.bfloat16, kind="ExternalInput")
    wl48 = nc.dram_tensor("wl48", [K * CIN, COUT], mybir.dt.bfloat16, kind="ExternalInput")
    bv = nc.dram_tensor("bv", [2 * COUT, 1], mybir.dt.float32, kind="ExternalInput")
    # full 64x64 (h,w) planes incl. garbage columns — contiguous stores;
    # the valid [.., :62, :62] crop happens on the host
    out = nc.dram_tensor(
        "out", [COUT, SO * PLANE], mybir.dt.float32, kind="ExternalOutput"
    )

    fp32 = mybir.dt.float32
    bf16 = mybir.dt.bfloat16
    AF = mybir.ActivationFunctionType

    with PatchedTileContext(nc) as tc:
        with (
            tc.tile_pool(name="const", bufs=1) as cpool,
            tc.tile_pool(name="xwina", bufs=2) as xapool,
            tc.tile_pool(name="xwin48", bufs=2) as x48pool,
            tc.tile_pool(name="eptmp", bufs=3) as epool,
            tc.tile_pool(name="outp", bufs=3) as opool,
            tc.tile_pool(name="psum", bufs=2, space="PSUM") as pspool,
        ):
            wla_sb = cpool.tile([128, K * COUT], bf16)
            wl48_sb = cpool.tile([K * CIN, COUT], bf16)
            bv_sb = cpool.tile([2 * COUT, 1], fp32)
            nc.sync.dma_start(wla_sb[:], wla[:])
            nc.sync.dma_start(wl48_sb[:], wl48[:])
            nc.sync.dma_start(bv_sb[:], bv[:])

            def body():
                for d0 in CHUNK_STARTS:
                    run_chunk(d0)

            def run_chunk(d0):
                ndp = min(CHUNK, SO - d0)  # output planes this chunk
                # single full-width DMA per window: 128 partitions = all ports
                xa = xapool.tile([128, WINA], bf16, tag="xa")
                nc.sync.dma_start(
                    xa[:], xr[:, d0 * PLANE : d0 * PLANE + WINA]
                )
                # tile48 window starts at d0+2 (only the kd=2,kw=2 tap reads it);
                # issued on the Pool/SWDGE queue to keep SP free
                x48 = x48pool.tile([K * CIN, WIN48], bf16, tag="x48")
                nc.gpsimd.dma_start(
                    x48[:],
                    xr48[:, (d0 + 2) * PLANE : (d0 + 2) * PLANE + WIN48],
                )

                for dl in range(ndp):
                    d = d0 + dl
                    # plane halves land on PSUM partition halves: p = 64*half+co
                    # pass-outer order: 8 consecutive matmuls share one lhsT
                    ps = pspool.tile([2 * COUT, HALF], fp32, tag="ps")
                    if "mm" in ablate:
                        nc.tensor.matmul(
                            ps[0:COUT, 0:512],
                            wla_sb[:, 0:COUT],
                            xa[:, 0:512],
                            start=True,
                            stop=True,
                        )
                    else:
                        # half innermost: consecutive matmuls alternate PE
                        # column groups (PSUM partitions 0:64 / 64:128) so the
                        # 32x32 sub-arrays can overlap execution
                        for kh in range(K):
                            for b in range(4):
                                for half in range(2):
                                    po = half * COUT
                                    col = half * HALF + b * 512
                                    oa = dl * PLANE + kh * S + col
                                    nc.tensor.matmul(
                                        ps[po : po + COUT, b * 512 : (b + 1) * 512],
                                        wla_sb[:, kh * COUT : (kh + 1) * COUT],
                                        xa[:, oa : oa + 512],
                                        start=(kh == 0),
                                        stop=False,
                                    )
                        for b in range(4):
                            for half in range(2):
                                po = half * COUT
                                col = half * HALF + b * 512
                                o48 = dl * PLANE + 2 + col
                                nc.tensor.matmul(
                                    ps[po : po + COUT, b * 512 : (b + 1) * 512],
                                    wl48_sb[:],
                                    x48[:, o48 : o48 + 512],
                                    start=False,
                                    stop=True,
                                )
                    # 128-partition epilogue: both halves in one ACT pass
                    t_sb = epool.tile([2 * COUT, HALF], fp32, tag="t")
                    o_sb = opool.tile([2 * COUT, HALF], fp32, tag="o")
                    if "act" not in ablate:
                        nc.scalar.activation(t_sb[:], ps[:], AF.Tanh)
                        nc.scalar.activation(
                            o_sb[:], t_sb[:], AF.Sigmoid, scale=bv_sb[:]
                        )
                    else:
                        nc.scalar.activation(o_sb[:, 0:512], ps[:, 0:512], AF.Tanh)
                    # two fully-contiguous stores per plane, split across the
                    # SP and ACT HWDGE queues (descriptor-light)
                    nc.sync.dma_start(
                        out[:, d * PLANE : d * PLANE + HALF], o_sb[0:COUT, :]
                    )
                    nc.scalar.dma_start(
                        out[:, d * PLANE + HALF : (d + 1) * PLANE],
                        o_sb[COUT : 2 * COUT, :],
                    )

            for _ in range(repeat):
                body()
    return nc


_NC_CACHE = None
LAST_RESULT = None


def kernel(x, weight, scale, bias):
    global _NC_CACHE, LAST_RESULT
    x = np.asarray(x, dtype=np.float32)
    weight = np.asarray(weight, dtype=np.float32)
    scale = np.asarray(scale, dtype=np.float32)
    bias = np.asarray(bias, dtype=np.float32)

    # fold scale into weights
    w_eff = weight * scale.reshape(COUT, 1, 1, 1, 1)  # [co, ci, kd, kh, kw]

    # wla: [128 = (kd,kw,ci) packed, kh*64 + co]
    wla = np.zeros((128, K * COUT), dtype=np.float32)
    wt = w_eff.transpose(2, 4, 1, 3, 0)  # [kd, kw, ci, kh, co]
    wla[0:96] = wt[0:2].reshape(96, K * COUT)
    wla[96:128] = wt[2, 0:2].reshape(32, K * COUT)
    wla = wla.astype(ml_dtypes.bfloat16)

    # wl48: [48 = (kh,ci), co] for tap (kd=2, kw=2)
    wl48 = np.ascontiguousarray(
        w_eff[:, :, 2, :, 2].transpose(2, 1, 0).reshape(K * CIN, COUT)
    ).astype(ml_dtypes.bfloat16)

    bv = np.ascontiguousarray(
        np.concatenate([bias.reshape(COUT, 1)] * 2, axis=0)
    )

    # host-side cast to bf16 + shift-replication into the matmul layouts
    xf = x.reshape(B, CIN, S * PLANE).astype(ml_dtypes.bfloat16)
    NTOT = S * PLANE
    xr = np.zeros((B, 128, XPLANES * PLANE), dtype=ml_dtypes.bfloat16)
    for kd in range(K):
        for kw in range(K):
            if kd == 2 and kw == 2:
                continue
            p = kd * 48 + kw * 16 if kd < 2 else 96 + kw * 16
            sh = kd * PLANE + kw
            xr[:, p : p + CIN, : NTOT - sh] = xf[:, :, sh:]
    xr48 = np.zeros((B, K * CIN, XPLANES * PLANE), dtype=ml_dtypes.bfloat16)
    for kh in range(K):
        sh = kh * S
        xr48[:, kh * CIN : (kh + 1) * CIN, : NTOT - sh] = xf[:, :, sh:]

    if _NC_CACHE is None:
        _NC_CACHE = build_nc()
    nc = _NC_CACHE

    in_maps = [
        {"xr": xr[c], "xr48": xr48[c], "wla": wla, "wl48": wl48, "bv": bv}
        for c in range(NCORES)
    ]
    res = run_bass_kernel_spmd(nc, in_maps, list(range(NCORES)))
    LAST_RESULT = res

    out = np.empty((B, COUT, SO, SO, SO), dtype=np.float32)
    for c in range(NCORES):
        full = res.results[c]["out"].reshape(COUT, SO, S, S)
        out[c] = full[:, :, :SO, :SO]
    return out
